# revision 4
# baseline (speedup 1.0000x reference)
"""Trainium2 Bass kernel for BaseGCN graph Laplacian (B=4, N=4096, C=3, k=20).

Math: reference computes L = I - D^{-1/2} A D^{-1/2} with A the one-hot
scatter of the k=20 nearest neighbours (euclidean, self included) per row.
top_k always returns exactly k distinct indices, so deg == k for every row
and L = I - A/k exactly: 0.95 on the diagonal, -0.05 at the 19 non-self
neighbour columns, 0 elsewhere.

Sharding: 8 cores; core = 2*b + half owns rows [half*2048, half*2048+2048)
of batch b and emits a (2048, 4096) f32 output slice.

Device algorithm per 128-row chunk:
  s[i,j] = -||x_i - x_j||^2 = 2<x_i,x_j> - sq_i - sq_j via a K=5 fp32
  matmul: lhsT rows [2x0,2x1,2x2,-1,-sq_i], rhs rows [x0,x1,x2,sq_j,1].
  Per-row top-20 threshold: segmented max8 over 16 segments of 256 columns
  -> 128 candidates/row, then 3 rounds of max8+match_replace on the
  candidates -> the 20th largest value T (exact unless one 256-segment
  holds >= 9 of a row's top-20; probability ~1e-4 over the whole problem).
  out = (s >= T) * (-1/k), plus identity added at the diagonal block
  (position fed per-core via the identc input so one NEFF serves all
  cores), then a 2 MB DMA to DRAM.
"""

import numpy as np

B, N, C = 4, 4096, 3
K = 20
P = 128                     # partition rows per chunk
ROWS = N // 2               # rows per core
NCHUNK = ROWS // P          # 16
NSEG = 16                   # max8 segments per row
SEG = N // NSEG             # 256
NEG = -1.0e30
# Match the reference's fl(dinv*dinv) rounding: dinv = fl(1/sqrt(20)) in f32.
_DINV = np.float32(1.0) / np.sqrt(np.float32(K))
VNEIGH = -float(np.float32(_DINV * _DINV))

_NC_CACHE = []


def _build_bass():
    import concourse.mybir as mybir
    import concourse.tile as tile
    from concourse import bacc

    f32 = mybir.dt.float32
    nc = bacc.Bacc("TRN2", debug=False, num_devices=8)
    rh = nc.dram_tensor("rh", (5, N), f32, kind="ExternalInput").ap()
    lh = nc.dram_tensor("lh", (5, ROWS), f32, kind="ExternalInput").ap()
    identc = nc.dram_tensor("identc", (P, 2 * P), f32, kind="ExternalInput").ap()
    outp = nc.dram_tensor("outp", (ROWS, N), f32, kind="ExternalOutput").ap()

    with tile.TileContext(nc) as tc:
        with (
            tc.tile_pool(name="const", bufs=1) as const_pool,
            tc.tile_pool(name="psum", bufs=2, space="PSUM") as psum_pool,
            tc.tile_pool(name="sbig", bufs=2) as s_pool,
            tc.tile_pool(name="small", bufs=2) as small_pool,
            tc.tile_pool(name="outt", bufs=2) as out_pool,
        ):
            rh_sb = const_pool.tile([5, N], f32)
            nc.sync.dma_start(rh_sb[:], rh)
            lh_sb = const_pool.tile([5, ROWS], f32)
            nc.sync.dma_start(lh_sb[:], lh)
            id_sb = const_pool.tile([P, 2 * P], f32)
            nc.sync.dma_start(id_sb[:], identc)

            for c in range(NCHUNK):
                s = s_pool.tile([P, N], f32, tag="s")
                for h in range(2):
                    ps = psum_pool.tile([P, N // 2], f32, tag="ps")
                    for t in range(4):
                        col = h * (N // 2) + t * 512
                        nc.tensor.matmul(
                            ps[:, t * 512:(t + 1) * 512],
                            lh_sb[:, c * P:(c + 1) * P],
                            rh_sb[:, col:col + 512],
                            start=True,
                            stop=True,
                        )
                    nc.scalar.activation(
                        s[:, h * (N // 2):(h + 1) * (N // 2)],
                        ps[:],
                        mybir.ActivationFunctionType.Copy,
                    )

                cand = small_pool.tile([P, NSEG * 8], f32, tag="cand")
                for g in range(NSEG):
                    nc.vector.max(
                        cand[:, g * 8:(g + 1) * 8], s[:, g * SEG:(g + 1) * SEG]
                    )
                m = small_pool.tile([P, 24], f32, tag="m")
                nc.vector.max(m[:, 0:8], cand[:])
                nc.vector.match_replace(cand[:], m[:, 0:8], cand[:], NEG)
                nc.vector.max(m[:, 8:16], cand[:])
                nc.vector.match_replace(cand[:], m[:, 8:16], cand[:], NEG)
                nc.vector.max(m[:, 16:24], cand[:])
                # 20th largest value overall = index 19 of the sorted 24

                ot = out_pool.tile([P, N], f32, tag="ot")
                nc.vector.tensor_scalar(
                    ot[:],
                    s[:],
                    m[:, 19:20],
                    VNEIGH,
                    op0=mybir.AluOpType.is_ge,
                    op1=mybir.AluOpType.mult,
                )
                # Diagonal block: rows c*P..c*P+P of this core map to global
                # columns c*P (half 0) or ROWS+c*P (half 1); identc carries
                # I at the half this core owns and zeros at the other.
                nc.vector.tensor_add(
                    ot[:, c * P:(c + 1) * P], ot[:, c * P:(c + 1) * P], id_sb[:, 0:P]
                )
                nc.vector.tensor_add(
                    ot[:, ROWS + c * P:ROWS + (c + 1) * P],
                    ot[:, ROWS + c * P:ROWS + (c + 1) * P],
                    id_sb[:, P:2 * P],
                )
                nc.sync.dma_start(outp[c * P:(c + 1) * P, :], ot[:])
    nc.compile()
    return nc


def _make_in_maps(x):
    eye = np.eye(P, dtype=np.float32)
    zero = np.zeros((P, P), dtype=np.float32)
    in_maps = []
    for core in range(8):
        b, half = divmod(core, 2)
        xb = x[b]                                            # (N, C)
        sq = (xb * xb).sum(axis=1, dtype=np.float32)
        rows = slice(half * ROWS, (half + 1) * ROWS)
        rh = np.empty((5, N), np.float32)
        rh[0:3] = xb.T
        rh[3] = sq
        rh[4] = 1.0
        lhs = np.empty((5, ROWS), np.float32)
        lhs[0:3] = 2.0 * xb[rows].T
        lhs[3] = -1.0
        lhs[4] = -sq[rows]
        identc = np.ascontiguousarray(
            np.concatenate([eye, zero] if half == 0 else [zero, eye], axis=1)
        )
        in_maps.append({"rh": rh, "lh": lhs, "identc": identc})
    return in_maps


def kernel(x, k):
    x = np.ascontiguousarray(np.asarray(x), dtype=np.float32)
    k = int(np.asarray(k))
    assert x.shape == (B, N, C), f"unexpected x shape {x.shape}"
    assert k == K, f"kernel compiled for k={K}, got {k}"

    from concourse.bass_utils import run_bass_kernel_spmd

    if not _NC_CACHE:
        _NC_CACHE.append(_build_bass())
    nc = _NC_CACHE[0]
    res = run_bass_kernel_spmd(nc, _make_in_maps(x), core_ids=list(range(8)))
    kernel.last_results = res
    out = np.empty((B, N, N), np.float32)
    for core in range(8):
        b, half = divmod(core, 2)
        out[b, half * ROWS:(half + 1) * ROWS] = res.results[core]["outp"]
    return out


# revision 9
# speedup vs baseline: 1.5271x; 1.5271x over previous
"""Trainium2 Bass kernel for BaseGCN graph Laplacian (B=4, N=4096, C=3, k=20).

Math: reference computes L = I - D^{-1/2} A D^{-1/2} with A the one-hot
scatter of the k=20 nearest neighbours (euclidean, self included) per row.
top_k always returns exactly k distinct indices, so deg == k for every row
and L = I - A/k exactly: 0.95 on the diagonal, -0.05 at the 19 non-self
neighbour columns, 0 elsewhere.

Sharding: 8 cores; core = 2*b + half owns rows [half*2048, half*2048+2048)
of batch b and emits a (2048, 4096) f32 output slice.

Device algorithm per 128-row chunk:
  s[i,j] = -||x_i - x_j||^2 = 2<x_i,x_j> - sq_i - sq_j via a K=24 bf16
  matmul (fp32 PE matmul runs at 4 cyc/row; bf16 at 1). Each fp32 operand
  is split into three bf16 limbs (hi/mid/lo, 24 mantissa bits total); the
  products kept (hh, hm, mh, mm, hl, lh per coordinate, plus 3-limb sq
  rows) are each exact in fp32, and dropped cross terms are ~2^-26 x^2 --
  the reconstruction error is at the same level as an fp32 einsum.
  Per-row top-20 threshold: segmented max8 over 16 segments of 256 columns
  -> 128 candidates/row, then 3 rounds of max8+match_replace on the
  candidates -> the 20th largest value T (exact unless one 256-segment
  holds >= 9 of a row's top-20; probability ~1e-4 over the whole problem).
  out = (s >= T) * (-1/k), plus identity added at the diagonal block
  (position fed per-core via the identc input so one NEFF serves all
  cores), then a 2 MB DMA to DRAM.
"""

import numpy as np

B, N, C = 4, 4096, 3
K = 20
P = 128                     # partition rows per chunk
ROWS = N // 2               # rows per core
NCHUNK = ROWS // P          # 16
NSEG = 16                   # max8 segments per row
SEG = N // NSEG             # 256
NEG = -1.0e30
# Match the reference's fl(dinv*dinv) rounding: dinv = fl(1/sqrt(20)) in f32.
_DINV = np.float32(1.0) / np.sqrt(np.float32(K))
VNEIGH = -float(np.float32(_DINV * _DINV))

_NC_CACHE = []


KMM = 24  # bf16-limb contraction depth


def _build_bass():
    import concourse.mybir as mybir
    import concourse.tile as tile
    from concourse import bacc

    f32 = mybir.dt.float32
    bf16 = mybir.dt.bfloat16
    nc = bacc.Bacc("TRN2", debug=False, num_devices=8)
    rh = nc.dram_tensor("rh", (KMM, N), bf16, kind="ExternalInput").ap()
    lh = nc.dram_tensor("lh", (KMM, ROWS), bf16, kind="ExternalInput").ap()
    identc = nc.dram_tensor("identc", (P, 2 * P), f32, kind="ExternalInput").ap()
    outp = nc.dram_tensor("outp", (ROWS, N), f32, kind="ExternalOutput").ap()

    with tile.TileContext(nc) as tc:
        with (
            tc.tile_pool(name="const", bufs=1) as const_pool,
            tc.tile_pool(name="psum", bufs=2, space="PSUM") as psum_pool,
            tc.tile_pool(name="sbig", bufs=2) as s_pool,
            tc.tile_pool(name="small", bufs=2) as small_pool,
            tc.tile_pool(name="outt", bufs=2) as out_pool,
        ):
            rh_sb = const_pool.tile([KMM, N], bf16)
            nc.sync.dma_start(rh_sb[:], rh)
            lh_sb = const_pool.tile([KMM, ROWS], bf16)
            nc.sync.dma_start(lh_sb[:], lh)
            id_sb = const_pool.tile([P, 2 * P], f32)
            nc.sync.dma_start(id_sb[:], identc)

            for c in range(NCHUNK):
                s = s_pool.tile([P, N], f32, tag="s")
                for h in range(2):
                    ps = psum_pool.tile([P, N // 2], f32, tag="ps")
                    for t in range(4):
                        col = h * (N // 2) + t * 512
                        nc.tensor.matmul(
                            ps[:, t * 512:(t + 1) * 512],
                            lh_sb[:, c * P:(c + 1) * P],
                            rh_sb[:, col:col + 512],
                            start=True,
                            stop=True,
                        )
                    nc.scalar.activation(
                        s[:, h * (N // 2):(h + 1) * (N // 2)],
                        ps[:],
                        mybir.ActivationFunctionType.Copy,
                    )

                cand = small_pool.tile([P, NSEG * 8], f32, tag="cand")
                for g in range(NSEG):
                    nc.vector.max(
                        cand[:, g * 8:(g + 1) * 8], s[:, g * SEG:(g + 1) * SEG]
                    )
                m = small_pool.tile([P, 24], f32, tag="m")
                nc.vector.max(m[:, 0:8], cand[:])
                nc.vector.match_replace(cand[:], m[:, 0:8], cand[:], NEG)
                nc.vector.max(m[:, 8:16], cand[:])
                nc.vector.match_replace(cand[:], m[:, 8:16], cand[:], NEG)
                nc.vector.max(m[:, 16:24], cand[:])
                # 20th largest value overall = index 19 of the sorted 24

                ot = out_pool.tile([P, N], f32, tag="ot")
                nc.vector.tensor_scalar(
                    ot[:],
                    s[:],
                    m[:, 19:20],
                    VNEIGH,
                    op0=mybir.AluOpType.is_ge,
                    op1=mybir.AluOpType.mult,
                )
                # Diagonal block: rows c*P..c*P+P of this core map to global
                # columns c*P (half 0) or ROWS+c*P (half 1); identc carries
                # I at the half this core owns and zeros at the other.
                # On GpSimd: it is otherwise idle and these are tiny.
                nc.gpsimd.tensor_add(
                    ot[:, c * P:(c + 1) * P], ot[:, c * P:(c + 1) * P], id_sb[:, 0:P]
                )
                nc.gpsimd.tensor_add(
                    ot[:, ROWS + c * P:ROWS + (c + 1) * P],
                    ot[:, ROWS + c * P:ROWS + (c + 1) * P],
                    id_sb[:, P:2 * P],
                )
                nc.sync.dma_start(outp[c * P:(c + 1) * P, :], ot[:])
    nc.compile()
    return nc


def _split3(v):
    """Split fp32 array into three bf16 limbs: v ~= h + m + l (24 bits)."""
    import ml_dtypes

    bf = ml_dtypes.bfloat16
    h = v.astype(bf)
    r = (v - h.astype(np.float32)).astype(np.float32)
    m = r.astype(bf)
    l = (r - m.astype(np.float32)).astype(bf)
    return h, m, l


def _make_in_maps(x):
    import ml_dtypes

    bf = ml_dtypes.bfloat16
    eye = np.eye(P, dtype=np.float32)
    zero = np.zeros((P, P), dtype=np.float32)
    in_maps = []
    for core in range(8):
        b, half = divmod(core, 2)
        xb = x[b]                                            # (N, C)
        sq = (xb * xb).sum(axis=1, dtype=np.float32)
        rows = slice(half * ROWS, (half + 1) * ROWS)
        rh = np.empty((KMM, N), bf)
        lhs = np.empty((KMM, ROWS), bf)
        for c in range(3):
            h, m, l = _split3(xb[:, c])
            h2 = (2.0 * h.astype(np.float32)).astype(bf)
            m2 = (2.0 * m.astype(np.float32)).astype(bf)
            l2 = (2.0 * l.astype(np.float32)).astype(bf)
            # product pairs (lhs, rhs): (2h,h) (2h,m) (2m,h) (2m,m) (2h,l) (2l,h)
            rh[6 * c + 0] = h
            rh[6 * c + 1] = m
            rh[6 * c + 2] = h
            rh[6 * c + 3] = m
            rh[6 * c + 4] = l
            rh[6 * c + 5] = h
            lhs[6 * c + 0] = h2[rows]
            lhs[6 * c + 1] = h2[rows]
            lhs[6 * c + 2] = m2[rows]
            lhs[6 * c + 3] = m2[rows]
            lhs[6 * c + 4] = h2[rows]
            lhs[6 * c + 5] = l2[rows]
        sh, sm, sl = _split3(sq)
        # -sq_j rows: lhs = -1, rhs = sq limbs
        rh[18], rh[19], rh[20] = sh, sm, sl
        lhs[18] = lhs[19] = lhs[20] = np.array(-1.0, bf)
        # -sq_i rows: lhs = -sq limbs, rhs = 1
        rh[21] = rh[22] = rh[23] = np.array(1.0, bf)
        lhs[21] = (-sh.astype(np.float32)).astype(bf)[rows]
        lhs[22] = (-sm.astype(np.float32)).astype(bf)[rows]
        lhs[23] = (-sl.astype(np.float32)).astype(bf)[rows]
        identc = np.ascontiguousarray(
            np.concatenate([eye, zero] if half == 0 else [zero, eye], axis=1)
        )
        in_maps.append({"rh": rh, "lh": lhs, "identc": identc})
    return in_maps


def kernel(x, k):
    x = np.ascontiguousarray(np.asarray(x), dtype=np.float32)
    k = int(np.asarray(k))
    assert x.shape == (B, N, C), f"unexpected x shape {x.shape}"
    assert k == K, f"kernel compiled for k={K}, got {k}"

    from concourse.bass_utils import run_bass_kernel_spmd

    if not _NC_CACHE:
        _NC_CACHE.append(_build_bass())
    nc = _NC_CACHE[0]
    res = run_bass_kernel_spmd(nc, _make_in_maps(x), core_ids=list(range(8)))
    kernel.last_results = res
    out = np.empty((B, N, N), np.float32)
    for core in range(8):
        b, half = divmod(core, 2)
        out[b, half * ROWS:(half + 1) * ROWS] = res.results[core]["outp"]
    return out
